# revision 17
# baseline (speedup 1.0000x reference)
"""Trainium2 Bass kernel for nn_Attn attention-context module.

Computation (per batch b):
    enc_att = enc @ W_enc + b_enc                      # [S, A]
    dec_att = dec @ W_dec + b_dec                      # [A]
    scores  = tanh(enc_att + dec_att) @ W_att + b_att  # [S]
    w       = softmax(mask(scores))                    # over S
    out     = sum_s w[s] * enc_att[s]                  # [A]

Strategy: data-parallel over batch across 8 NeuronCores (4 batches each),
weights replicated. enc is pre-transposed and cast to bf16 on the host
(layout [b, t, i, p, n] = enc[b, t*TT+n, i*128+p]) so each core streams
contiguous, already-transposed bf16 tiles straight from HBM -- no on-device
transpose pass and half the HBM traffic of fp32. Per core:
  - PE computes enc_attT chunks [A-chunk(128), TT tok] in PSUM (bf16 in,
    fp32 acc)
  - ACT applies tanh (bf16 out) with per-partition bias = dec_att + b_enc,
    and copies raw enc_att to SBUF fp32 for the context accumulation
  - scores via small bf16 PE matmuls with lhsT = W_att chunks; the mask is
    folded in as a -32768*mask K=1 matmul term (exp then underflows to 0)
  - softmax without max-subtraction (|scores| <= ||W_att||_1 ~ 51, exp can't
    overflow fp32; b_att cancels in the softmax so it is dropped)
  - context accumulated per tile with fused DVE multiply+row-sum
    (scalar_tensor_tensor with accum_out) against a broadcast row of softmax
    numerators; normalization and b_enc are applied once per batch
"""

import os
import sys

import numpy as np

for _p in ("/opt/trn_rl_repo", "/root/.axon_site/_ro/trn_rl_repo"):
    if os.path.isdir(_p) and _p not in sys.path:
        sys.path.append(_p)

import concourse.bass as bass
import bass_rust
import concourse.mybir as mybir
from concourse import tile
from concourse.bass_utils import run_bass_kernel_spmd

P = 128
E = 1024          # 2*HIDDEN
A = 512           # ATT
HID = 512
S = 2048
B = 32
NCORES = 8
BLOC = B // NCORES           # 4 batches per core
TT = 512                     # tokens per tile
NT = S // TT                 # 4 tiles per batch
NE = E // P                  # 8 E-chunks
NA = A // P                  # 4 A-chunks
NK = TT // P                 # 4 token blocks per tile

f32 = mybir.dt.float32
bf16 = mybir.dt.bfloat16
u8 = mybir.dt.uint8

_CACHE = {}


def _split_multiwaits(nc):
    """This toolchain's walrus encodes at most 1 sync-wait per instruction
    (2 for EventSemaphore). Hoist extra waits onto pure-wait EventSemaphore
    instructions inserted immediately before the offender (same engine), which
    preserves semantics exactly."""
    n_split = 0
    uid = 0
    for fn in nc.m.functions:
        for blk in fn.blocks:
            new_insts = []
            for inst in blk.instructions:
                cap = 2 if type(inst).__name__ == "InstEventSemaphore" else 1
                si = inst.sync_info
                waits = list(si.on_wait) if si is not None and si.on_wait else []
                if len(waits) > cap:
                    extra, keep = waits[:-cap], waits[-cap:]
                    for i in range(0, len(extra), 2):
                        uid += 1
                        new_insts.append(bass_rust.InstEventSemaphore(
                            name=f"splitwait_{uid}_{inst.name}",
                            engine=inst.engine,
                            ins=[],
                            outs=[],
                            sync_info=bass_rust.SyncInfo(
                                on_wait=list(extra[i:i + 2]), on_update=[]),
                        ))
                        n_split += 1
                    si.on_wait = keep
                new_insts.append(inst)
            blk.instructions[:] = new_insts
    return n_split


def build(encbufs=3, reps=1):
    nc = bass.Bass("TRN2", debug=False)
    # host-pre-transposed bf16 enc: [b, t, i, p, n] = enc[b, t*TT+n, i*P+p]
    encT = nc.dram_tensor("encT", [BLOC, NT, NE, P, TT], bf16,
                          kind="ExternalInput")
    dec = nc.dram_tensor("dec", [BLOC, HID], f32, kind="ExternalInput")
    masks = nc.dram_tensor("masks", [BLOC, S], u8, kind="ExternalInput")
    w_enc = nc.dram_tensor("w_enc", [E, A], f32, kind="ExternalInput")
    b_enc = nc.dram_tensor("b_enc", [A], f32, kind="ExternalInput")
    w_dec = nc.dram_tensor("w_dec", [HID, A], f32, kind="ExternalInput")
    b_dec = nc.dram_tensor("b_dec", [A], f32, kind="ExternalInput")
    w_att = nc.dram_tensor("w_att", [A], f32, kind="ExternalInput")
    out = nc.dram_tensor("out", [BLOC, A], f32, kind="ExternalOutput")

    Tanh = mybir.ActivationFunctionType.Tanh
    Exp = mybir.ActivationFunctionType.Exp
    Copy = mybir.ActivationFunctionType.Copy
    add = mybir.AluOpType.add
    mult = mybir.AluOpType.mult
    X = mybir.AxisListType.X

    with tile.TileContext(nc) as tc:
        with (
            tc.tile_pool(name="const", bufs=1) as cp,
            tc.tile_pool(name="encT", bufs=encbufs) as encp,
            tc.tile_pool(name="tanh", bufs=4) as tanhp,
            tc.tile_pool(name="ea", bufs=4) as eap,
            tc.tile_pool(name="small", bufs=3) as smp,
            tc.tile_pool(name="attps", bufs=2, space="PSUM") as attp,
            tc.tile_pool(name="scps", bufs=2, space="PSUM") as scp,
            tc.tile_pool(name="pbps", bufs=2, space="PSUM") as pbp,
        ):
            # ---------------- one-time prep ----------------
            # W_enc bf16: [e' part, (i, a)] for e = i*128 + e'
            wsb = cp.tile([P, NE * A], bf16, tag="wsb")
            nc.gpsimd.dma_start(
                wsb[:].rearrange("p (i a) -> p i a", i=NE),
                w_enc.ap().rearrange("(i p) a -> p i a", p=P))
            # W_dec f32: [h' part, (i, a)] for h = i*128 + h'
            # (one-time loads ride the vector/scalar queues so the sync queue
            # is free for the first encT tile)
            wdsb = cp.tile([P, (HID // P) * A], f32, tag="wdsb")
            nc.scalar.dma_start(
                wdsb[:].rearrange("p (i a) -> p i a", i=HID // P),
                w_dec.ap().rearrange("(i p) a -> p i a", p=P))
            # W_att bf16 column chunks [a' part, j]
            wasb = cp.tile([P, NA], bf16, tag="wasb")
            nc.gpsimd.dma_start(wasb[:], w_att.ap().rearrange("(j p) -> p j", p=P))
            # biases as column chunks [a' part, j]
            besb = cp.tile([P, NA], f32, tag="besb")
            nc.scalar.dma_start(besb[:], b_enc.ap().rearrange("(j p) -> p j", p=P))
            bdsb = cp.tile([P, NA], f32, tag="bdsb")
            nc.scalar.dma_start(bdsb[:], b_dec.ap().rearrange("(j p) -> p j", p=P))
            bbsb = cp.tile([P, NA], f32, tag="bbsb")
            nc.vector.tensor_tensor(bbsb[:], besb[:], bdsb[:], op=add)
            # decoder_hidden transposed [h' part, (hc, b)] (tiny strided load)
            dhT = cp.tile([P, (HID // P) * BLOC], f32, tag="dhT")
            with nc.allow_non_contiguous_dma(reason="8KB one-time transposed load"):
                for hc in range(HID // P):
                    nc.scalar.dma_start(
                        dhT[:, hc * BLOC:(hc + 1) * BLOC],
                        dec.ap()[:, hc * P:(hc + 1) * P].rearrange("b p -> p b"))
            # masks, whole core's worth: [1, BLOC*S] u8 -> bf16
            msku = cp.tile([1, BLOC * S], u8, tag="msku")
            nc.scalar.dma_start(msku[:], masks.ap().rearrange("b s -> (b s)")[None, :])
            mskf = cp.tile([1, BLOC * S], bf16, tag="mskf")
            nc.vector.tensor_copy(mskf[:], msku[:])
            # ones row (for partition broadcast via K=1 matmul)
            ones = cp.tile([1, P], bf16, tag="ones")
            nc.vector.memset(ones[:], 1.0)
            onesf = cp.tile([1, P], f32, tag="onesf")
            nc.vector.memset(onesf[:], 1.0)
            # mask weight for folding -32768*mask into the scores matmul
            m30 = cp.tile([1, 1], bf16, tag="m30")
            nc.vector.memset(m30[:], -32768.0)

            # dec_attT + bias columns: bias_sb[a', j*BLOC + b]
            bias_sb = cp.tile([P, NA * BLOC], f32, tag="bias_sb")
            for j in range(NA):
                pd = pbp.tile([P, BLOC], f32, tag="pb")
                for hc in range(HID // P):
                    nc.tensor.matmul(
                        pd[:],
                        lhsT=wdsb[:, hc * A + j * P: hc * A + (j + 1) * P],
                        rhs=dhT[:, hc * BLOC:(hc + 1) * BLOC],
                        start=(hc == 0), stop=(hc == HID // P - 1))
                nc.vector.tensor_scalar(
                    out=bias_sb[:, j * BLOC:(j + 1) * BLOC], in0=pd[:],
                    scalar1=bbsb[:, j:j + 1], scalar2=None, op0=add)

            # persistent accumulators
            ctxp = cp.tile([P, NA * NT], f32, tag="ctxp")       # per (A-chunk, tile) partial
            ctxs = cp.tile([P, NA], f32, tag="ctxs")
            dens = cp.tile([1, BLOC * NT], f32, tag="dens")     # per-tile denominators
            dent = cp.tile([1, BLOC], f32, tag="dent")
            rec = cp.tile([1, BLOC], f32, tag="rec")
            outsb = cp.tile([P, NA * BLOC], f32, tag="outsb")

            # ---------------- main loop ----------------
            # Per-tile epilogues (scores -> softmax -> context) are deferred
            # until after the NEXT tile's main matmuls are emitted, so the PE
            # queue always has dense main-matmul work between epilogue stalls.
            pending = []

            def tile_epilogue(b, t, ea_sb, tanh_sb):
                bt = b * NT + t
                # scores [1, TT]; mask folded in as a K=1 matmul term
                sc = scp.tile([1, TT], f32, tag="sc")
                for j in range(NA):
                    nc.tensor.matmul(
                        sc[:], lhsT=wasb[:, j:j + 1],
                        rhs=tanh_sb[:, j * TT:(j + 1) * TT],
                        start=(j == 0), stop=False)
                nc.tensor.matmul(
                    sc[:], lhsT=m30[:],
                    rhs=mskf[0:1, (b * S + t * TT):(b * S + (t + 1) * TT)],
                    start=False, stop=True)
                p_sb = smp.tile([1, TT], bf16, tag="p_sb")
                nc.scalar.activation(
                    p_sb[:], sc[:], Exp,
                    accum_out=dens[0:1, bt:bt + 1])
                # broadcast p across partitions via K=1 matmul
                pb = pbp.tile([P, TT], f32, tag="pb")
                nc.tensor.matmul(pb[:], lhsT=ones[:], rhs=p_sb[:])
                # fused context accumulation per A-chunk:
                # accum_out = sum_t(pb * ea) per partition
                waste = smp.tile([P, TT], bf16, tag="waste")
                for j in range(NA):
                    nc.vector.scalar_tensor_tensor(
                        out=waste[:],
                        in0=pb[:], scalar=1.0, in1=ea_sb[:, j * TT:(j + 1) * TT],
                        op0=mult, op1=mult,
                        accum_out=ctxp[:, j * NT + t:j * NT + t + 1])

            def batch_epilogue(b):
                # out[b] = ctx/den + b_enc
                nc.vector.reduce_sum(
                    dent[0:1, b:b + 1], dens[0:1, b * NT:(b + 1) * NT], axis=X)
                nc.vector.reciprocal(rec[0:1, b:b + 1], dent[0:1, b:b + 1])
                rb = pbp.tile([P, TT], f32, tag="pb")
                nc.tensor.matmul(rb[:, 0:1], lhsT=onesf[:], rhs=rec[0:1, b:b + 1])
                nc.vector.reduce_sum(
                    ctxs[:], ctxp[:].rearrange("p (j t) -> p j t", j=NA), axis=X)
                nc.vector.scalar_tensor_tensor(
                    out=outsb[:, b * NA:(b + 1) * NA],
                    in0=ctxs[:], scalar=rb[:, 0:1], in1=besb[:], op0=mult, op1=add)
                nc.gpsimd.dma_start(
                    out.ap()[b].rearrange("(j p) -> p j", p=P),
                    outsb[:, b * NA:(b + 1) * NA])

            for b, t in [(bb_, tt_) for _ in range(reps)
                         for bb_ in range(BLOC) for tt_ in range(NT)]:
                if True:
                    # load pre-transposed tile: encTt[e', (i, n)]
                    encTt = encp.tile([P, NE * TT], bf16, tag="encT")
                    nc.sync.dma_start(
                        encTt[:].rearrange("p (i n) -> p i n", i=NE),
                        encT.ap()[b, t].rearrange("i p n -> p i n"))

                    tanh_sb = tanhp.tile([P, NA * TT], bf16, tag="tanh")
                    ea_sb = eap.tile([P, NA * TT], bf16, tag="ea")
                    for jp in range(NA // 2):
                        att = attp.tile([P, 2 * TT], f32, tag="att")
                        for half in range(2):
                            j = jp * 2 + half
                            for i in range(NE):
                                nc.tensor.matmul(
                                    att[:, half * TT:(half + 1) * TT],
                                    lhsT=wsb[:, i * A + j * P: i * A + (j + 1) * P],
                                    rhs=encTt[:, i * TT:(i + 1) * TT],
                                    start=(i == 0), stop=(i == NE - 1))
                            nc.scalar.activation(
                                tanh_sb[:, j * TT:(j + 1) * TT],
                                att[:, half * TT:(half + 1) * TT],
                                Tanh, bias=bias_sb[:, j * BLOC + b: j * BLOC + b + 1])
                            # copy enc_att out of PSUM (frees the bank for the
                            # next tile's matmuls): ACT for the first pair,
                            # DVE for the second, to balance engine load.
                            if jp == 0:
                                nc.scalar.activation(
                                    ea_sb[:, j * TT:(j + 1) * TT],
                                    att[:, half * TT:(half + 1) * TT], Copy)
                            else:
                                nc.vector.tensor_copy(
                                    ea_sb[:, j * TT:(j + 1) * TT],
                                    att[:, half * TT:(half + 1) * TT])

                    while len(pending) >= 3:
                        pending.pop(0)()
                    pending.append(lambda b=b, t=t, a=ea_sb, ts=tanh_sb: tile_epilogue(b, t, a, ts))
                    if t == NT - 1:
                        # batch epilogue rides the deferred queue too, so the
                        # next batch's main matmuls keep the PE fed while this
                        # batch's softmax/context tail drains
                        pending.append(lambda b=b: batch_epilogue(b))
            for fn_ in pending:
                fn_()

    n = _split_multiwaits(nc)
    if os.environ.get("KERNEL_DEBUG"):
        print(f"[kernel] split {n} extra waits", file=sys.stderr)
    return nc


def _get_nc():
    if "nc" not in _CACHE:
        _CACHE["nc"] = build()
    return _CACHE["nc"]


def host_prep(enc_output, decoder_hidden, masks, W_enc, b_enc, W_dec, b_dec,
              W_att, b_att=None, **kwargs):
    """Shard + lay out inputs for the 8 cores. enc is cast to bf16 and
    pre-transposed to [b, t, i, p, n] = enc[b, t*TT+n, i*P+p] so the device
    streams contiguous already-transposed tiles."""
    import ml_dtypes

    enc_output = np.asarray(enc_output, dtype=np.float32)
    decoder_hidden = np.asarray(decoder_hidden, dtype=np.float32)
    masks_u8 = np.ascontiguousarray(np.asarray(masks).reshape(B, S)).view(np.uint8)
    # [B, S, E] -> [B, NT, TT, NE, P] -> [B, NT, NE, P, TT]
    encT = np.ascontiguousarray(
        enc_output.reshape(B, NT, TT, NE, P).transpose(0, 1, 3, 4, 2)
    ).astype(ml_dtypes.bfloat16)
    shared = {
        "w_enc": np.asarray(W_enc, dtype=np.float32),
        "b_enc": np.asarray(b_enc, dtype=np.float32).reshape(A),
        "w_dec": np.asarray(W_dec, dtype=np.float32),
        "b_dec": np.asarray(b_dec, dtype=np.float32).reshape(A),
        "w_att": np.asarray(W_att, dtype=np.float32).reshape(A),
    }
    in_maps = []
    for c in range(NCORES):
        sl = slice(c * BLOC, (c + 1) * BLOC)
        in_maps.append({
            "encT": encT[sl],
            "dec": decoder_hidden[sl],
            "masks": masks_u8[sl],
            **shared,
        })
    return in_maps


def kernel(enc_output, decoder_hidden, masks, W_enc, b_enc, W_dec, b_dec,
           W_att, b_att, **kwargs):
    # b_att shifts every score equally -> cancels in softmax; output does not
    # depend on it, so it is not shipped to the device.
    in_maps = host_prep(enc_output, decoder_hidden, masks, W_enc, b_enc,
                        W_dec, b_dec, W_att, b_att)
    res = run_bass_kernel_spmd(_get_nc(), in_maps, core_ids=list(range(NCORES)))
    return np.concatenate([res.results[c]["out"] for c in range(NCORES)], axis=0)


# revision 18
# speedup vs baseline: 1.2608x; 1.2608x over previous
"""Trainium2 Bass kernel for nn_Attn attention-context module.

Computation (per batch b):
    enc_att = enc @ W_enc + b_enc                      # [S, A]
    dec_att = dec @ W_dec + b_dec                      # [A]
    scores  = tanh(enc_att + dec_att) @ W_att + b_att  # [S]
    w       = softmax(mask(scores))                    # over S
    out     = sum_s w[s] * enc_att[s]                  # [A]

Strategy: data-parallel over batch across 8 NeuronCores (4 batches each),
weights replicated. enc is pre-transposed and cast to bf16 on the host
(layout [b, t, i, p, n] = enc[b, t*TT+n, i*128+p]) so each core streams
contiguous, already-transposed bf16 tiles straight from HBM -- no on-device
transpose pass and half the HBM traffic of fp32. Per core:
  - PE computes enc_attT chunks [A-chunk(128), TT tok] in PSUM (bf16 in,
    fp32 acc)
  - ACT applies tanh (bf16 out) with per-partition bias = dec_att + b_enc,
    and copies raw enc_att to SBUF fp32 for the context accumulation
  - scores via small bf16 PE matmuls with lhsT = W_att chunks; the mask is
    folded in as a -32768*mask K=1 matmul term (exp then underflows to 0)
  - softmax without max-subtraction (|scores| <= ||W_att||_1 ~ 51, exp can't
    overflow fp32; b_att cancels in the softmax so it is dropped)
  - context accumulated per tile with fused DVE multiply+row-sum
    (scalar_tensor_tensor with accum_out) against a broadcast row of softmax
    numerators; normalization and b_enc are applied once per batch
"""

import os
import sys

import numpy as np

for _p in ("/opt/trn_rl_repo", "/root/.axon_site/_ro/trn_rl_repo"):
    if os.path.isdir(_p) and _p not in sys.path:
        sys.path.append(_p)

import concourse.bass as bass
import bass_rust
import concourse.mybir as mybir
from concourse import tile
from concourse.bass_utils import run_bass_kernel_spmd

P = 128
E = 1024          # 2*HIDDEN
A = 512           # ATT
HID = 512
S = 2048
B = 32
NCORES = 8
BLOC = B // NCORES           # 4 batches per core
TT = 512                     # tokens per tile
NT = S // TT                 # 4 tiles per batch
NE = E // P                  # 8 E-chunks
NA = A // P                  # 4 A-chunks
NK = TT // P                 # 4 token blocks per tile

f32 = mybir.dt.float32
bf16 = mybir.dt.bfloat16
u8 = mybir.dt.uint8

_CACHE = {}


def _split_multiwaits(nc):
    """This toolchain's walrus encodes at most 1 sync-wait per instruction
    (2 for EventSemaphore). Hoist extra waits onto pure-wait EventSemaphore
    instructions inserted immediately before the offender (same engine), which
    preserves semantics exactly."""
    n_split = 0
    uid = 0
    for fn in nc.m.functions:
        for blk in fn.blocks:
            new_insts = []
            for inst in blk.instructions:
                cap = 2 if type(inst).__name__ == "InstEventSemaphore" else 1
                si = inst.sync_info
                waits = list(si.on_wait) if si is not None and si.on_wait else []
                if len(waits) > cap:
                    extra, keep = waits[:-cap], waits[-cap:]
                    for i in range(0, len(extra), 2):
                        uid += 1
                        new_insts.append(bass_rust.InstEventSemaphore(
                            name=f"splitwait_{uid}_{inst.name}",
                            engine=inst.engine,
                            ins=[],
                            outs=[],
                            sync_info=bass_rust.SyncInfo(
                                on_wait=list(extra[i:i + 2]), on_update=[]),
                        ))
                        n_split += 1
                    si.on_wait = keep
                new_insts.append(inst)
            blk.instructions[:] = new_insts
    return n_split


def build(encbufs=3, reps=1):
    nc = bass.Bass("TRN2", debug=False)
    # host-pre-transposed bf16 enc: [b, t, i, p, n] = enc[b, t*TT+n, i*P+p]
    encT = nc.dram_tensor("encT", [BLOC, NT, NE, P, TT], bf16,
                          kind="ExternalInput")
    dec = nc.dram_tensor("dec", [BLOC, HID], f32, kind="ExternalInput")
    masks = nc.dram_tensor("masks", [BLOC, S], u8, kind="ExternalInput")
    w_enc = nc.dram_tensor("w_enc", [E, A], f32, kind="ExternalInput")
    b_enc = nc.dram_tensor("b_enc", [A], f32, kind="ExternalInput")
    w_dec = nc.dram_tensor("w_dec", [HID, A], f32, kind="ExternalInput")
    b_dec = nc.dram_tensor("b_dec", [A], f32, kind="ExternalInput")
    w_att = nc.dram_tensor("w_att", [A], f32, kind="ExternalInput")
    out = nc.dram_tensor("out", [BLOC, A], f32, kind="ExternalOutput")

    Tanh = mybir.ActivationFunctionType.Tanh
    Exp = mybir.ActivationFunctionType.Exp
    Copy = mybir.ActivationFunctionType.Copy
    add = mybir.AluOpType.add
    mult = mybir.AluOpType.mult
    X = mybir.AxisListType.X

    with tile.TileContext(nc) as tc:
        with (
            tc.tile_pool(name="const", bufs=1) as cp,
            tc.tile_pool(name="encT", bufs=encbufs) as encp,
            tc.tile_pool(name="tanh", bufs=4) as tanhp,
            tc.tile_pool(name="ea", bufs=4) as eap,
            tc.tile_pool(name="small", bufs=3) as smp,
            tc.tile_pool(name="attps", bufs=2, space="PSUM") as attp,
            tc.tile_pool(name="scps", bufs=2, space="PSUM") as scp,
            tc.tile_pool(name="pbps", bufs=2, space="PSUM") as pbp,
        ):
            # ---------------- one-time prep ----------------
            # W_enc bf16: [e' part, (i, a)] for e = i*128 + e'
            wsb = cp.tile([P, NE * A], bf16, tag="wsb")
            nc.gpsimd.dma_start(
                wsb[:].rearrange("p (i a) -> p i a", i=NE),
                w_enc.ap().rearrange("(i p) a -> p i a", p=P))
            # W_dec f32: [h' part, (i, a)] for h = i*128 + h'
            # (one-time loads ride the vector/scalar queues so the sync queue
            # is free for the first encT tile)
            wdsb = cp.tile([P, (HID // P) * A], f32, tag="wdsb")
            nc.scalar.dma_start(
                wdsb[:].rearrange("p (i a) -> p i a", i=HID // P),
                w_dec.ap().rearrange("(i p) a -> p i a", p=P))
            # W_att bf16 column chunks [a' part, j]
            wasb = cp.tile([P, NA], bf16, tag="wasb")
            nc.gpsimd.dma_start(wasb[:], w_att.ap().rearrange("(j p) -> p j", p=P))
            # biases as column chunks [a' part, j]
            besb = cp.tile([P, NA], f32, tag="besb")
            nc.scalar.dma_start(besb[:], b_enc.ap().rearrange("(j p) -> p j", p=P))
            bdsb = cp.tile([P, NA], f32, tag="bdsb")
            nc.scalar.dma_start(bdsb[:], b_dec.ap().rearrange("(j p) -> p j", p=P))
            bbsb = cp.tile([P, NA], f32, tag="bbsb")
            nc.vector.tensor_tensor(bbsb[:], besb[:], bdsb[:], op=add)
            # decoder_hidden transposed [h' part, (hc, b)] (tiny strided load)
            dhT = cp.tile([P, (HID // P) * BLOC], f32, tag="dhT")
            with nc.allow_non_contiguous_dma(reason="8KB one-time transposed load"):
                for hc in range(HID // P):
                    nc.scalar.dma_start(
                        dhT[:, hc * BLOC:(hc + 1) * BLOC],
                        dec.ap()[:, hc * P:(hc + 1) * P].rearrange("b p -> p b"))
            # masks, whole core's worth: [1, BLOC*S] u8 -> bf16
            msku = cp.tile([1, BLOC * S], u8, tag="msku")
            nc.scalar.dma_start(msku[:], masks.ap().rearrange("b s -> (b s)")[None, :])
            mskf = cp.tile([1, BLOC * S], bf16, tag="mskf")
            nc.vector.tensor_copy(mskf[:], msku[:])
            # ones row (for partition broadcast via K=1 matmul)
            ones = cp.tile([1, P], bf16, tag="ones")
            nc.vector.memset(ones[:], 1.0)
            onesf = cp.tile([1, P], f32, tag="onesf")
            nc.vector.memset(onesf[:], 1.0)
            # mask weight for folding -32768*mask into the scores matmul
            m30 = cp.tile([1, 1], bf16, tag="m30")
            nc.vector.memset(m30[:], -32768.0)

            # dec_attT + bias columns: bias_sb[a', j*BLOC + b]
            bias_sb = cp.tile([P, NA * BLOC], f32, tag="bias_sb")
            for j in range(NA):
                pd = pbp.tile([P, BLOC], f32, tag="pb")
                for hc in range(HID // P):
                    nc.tensor.matmul(
                        pd[:],
                        lhsT=wdsb[:, hc * A + j * P: hc * A + (j + 1) * P],
                        rhs=dhT[:, hc * BLOC:(hc + 1) * BLOC],
                        start=(hc == 0), stop=(hc == HID // P - 1))
                nc.vector.tensor_scalar(
                    out=bias_sb[:, j * BLOC:(j + 1) * BLOC], in0=pd[:],
                    scalar1=bbsb[:, j:j + 1], scalar2=None, op0=add)

            # persistent accumulators
            ctxp = cp.tile([P, NA * NT], f32, tag="ctxp")       # per (A-chunk, tile) partial
            ctxs = cp.tile([P, NA], f32, tag="ctxs")
            dens = cp.tile([1, BLOC * NT], f32, tag="dens")     # per-tile denominators
            dent = cp.tile([1, BLOC], f32, tag="dent")
            rec = cp.tile([1, BLOC], f32, tag="rec")
            outsb = cp.tile([P, NA * BLOC], f32, tag="outsb")

            # ---------------- main loop ----------------
            # Per-tile epilogues (scores -> softmax -> context) are deferred
            # until after the NEXT tile's main matmuls are emitted, so the PE
            # queue always has dense main-matmul work between epilogue stalls.
            pending = []

            def tile_epilogue(b, t, ea_sb, tanh_sb):
                bt = b * NT + t
                # scores [1, TT]; mask folded in as a K=1 matmul term
                sc = scp.tile([1, TT], f32, tag="sc")
                for j in range(NA):
                    nc.tensor.matmul(
                        sc[:], lhsT=wasb[:, j:j + 1],
                        rhs=tanh_sb[:, j * TT:(j + 1) * TT],
                        start=(j == 0), stop=False)
                nc.tensor.matmul(
                    sc[:], lhsT=m30[:],
                    rhs=mskf[0:1, (b * S + t * TT):(b * S + (t + 1) * TT)],
                    start=False, stop=True)
                p_sb = smp.tile([1, TT], bf16, tag="p_sb")
                nc.scalar.activation(
                    p_sb[:], sc[:], Exp,
                    accum_out=dens[0:1, bt:bt + 1])
                # broadcast p across partitions via K=1 matmul
                pb = pbp.tile([P, TT], f32, tag="pb")
                nc.tensor.matmul(pb[:], lhsT=ones[:], rhs=p_sb[:])
                # fused context accumulation per A-chunk:
                # accum_out = sum_t(pb * ea) per partition
                waste = smp.tile([P, TT], bf16, tag="waste")
                for j in range(NA):
                    nc.vector.scalar_tensor_tensor(
                        out=waste[:],
                        in0=pb[:], scalar=1.0, in1=ea_sb[:, j * TT:(j + 1) * TT],
                        op0=mult, op1=mult,
                        accum_out=ctxp[:, j * NT + t:j * NT + t + 1])

            def batch_epilogue(b):
                # out[b] = ctx/den + b_enc
                nc.vector.reduce_sum(
                    dent[0:1, b:b + 1], dens[0:1, b * NT:(b + 1) * NT], axis=X)
                nc.vector.reciprocal(rec[0:1, b:b + 1], dent[0:1, b:b + 1])
                rb = pbp.tile([P, TT], f32, tag="pb")
                nc.tensor.matmul(rb[:, 0:1], lhsT=onesf[:], rhs=rec[0:1, b:b + 1])
                nc.vector.reduce_sum(
                    ctxs[:], ctxp[:].rearrange("p (j t) -> p j t", j=NA), axis=X)
                nc.vector.scalar_tensor_tensor(
                    out=outsb[:, b * NA:(b + 1) * NA],
                    in0=ctxs[:], scalar=rb[:, 0:1], in1=besb[:], op0=mult, op1=add)
                nc.gpsimd.dma_start(
                    out.ap()[b].rearrange("(j p) -> p j", p=P),
                    outsb[:, b * NA:(b + 1) * NA])

            for b, t in [(bb_, tt_) for _ in range(reps)
                         for bb_ in range(BLOC) for tt_ in range(NT)]:
                if True:
                    # load pre-transposed tile: encTt[e', (i, n)]
                    encTt = encp.tile([P, NE * TT], bf16, tag="encT")
                    nc.sync.dma_start(
                        encTt[:].rearrange("p (i n) -> p i n", i=NE),
                        encT.ap()[b, t].rearrange("i p n -> p i n"))

                    tanh_sb = tanhp.tile([P, NA * TT], bf16, tag="tanh")
                    ea_sb = eap.tile([P, NA * TT], f32, tag="ea")
                    for jp in range(NA // 2):
                        att = attp.tile([P, 2 * TT], f32, tag="att")
                        for half in range(2):
                            j = jp * 2 + half
                            for i in range(NE):
                                nc.tensor.matmul(
                                    att[:, half * TT:(half + 1) * TT],
                                    lhsT=wsb[:, i * A + j * P: i * A + (j + 1) * P],
                                    rhs=encTt[:, i * TT:(i + 1) * TT],
                                    start=(i == 0), stop=(i == NE - 1))
                            nc.scalar.activation(
                                tanh_sb[:, j * TT:(j + 1) * TT],
                                att[:, half * TT:(half + 1) * TT],
                                Tanh, bias=bias_sb[:, j * BLOC + b: j * BLOC + b + 1])
                            # copy enc_att out of PSUM (frees the bank for the
                            # next tile's matmuls): ACT for the first pair,
                            # DVE for the second, to balance engine load.
                            if jp == 0:
                                nc.scalar.activation(
                                    ea_sb[:, j * TT:(j + 1) * TT],
                                    att[:, half * TT:(half + 1) * TT], Copy)
                            else:
                                nc.vector.tensor_copy(
                                    ea_sb[:, j * TT:(j + 1) * TT],
                                    att[:, half * TT:(half + 1) * TT])

                    while len(pending) >= 3:
                        pending.pop(0)()
                    pending.append(lambda b=b, t=t, a=ea_sb, ts=tanh_sb: tile_epilogue(b, t, a, ts))
                    if t == NT - 1:
                        # batch epilogue rides the deferred queue too, so the
                        # next batch's main matmuls keep the PE fed while this
                        # batch's softmax/context tail drains
                        pending.append(lambda b=b: batch_epilogue(b))
            for fn_ in pending:
                fn_()

    n = _split_multiwaits(nc)
    if os.environ.get("KERNEL_DEBUG"):
        print(f"[kernel] split {n} extra waits", file=sys.stderr)
    return nc


def _get_nc():
    if "nc" not in _CACHE:
        _CACHE["nc"] = build()
    return _CACHE["nc"]


def host_prep(enc_output, decoder_hidden, masks, W_enc, b_enc, W_dec, b_dec,
              W_att, b_att=None, **kwargs):
    """Shard + lay out inputs for the 8 cores. enc is cast to bf16 and
    pre-transposed to [b, t, i, p, n] = enc[b, t*TT+n, i*P+p] so the device
    streams contiguous already-transposed tiles."""
    import ml_dtypes

    enc_output = np.asarray(enc_output, dtype=np.float32)
    decoder_hidden = np.asarray(decoder_hidden, dtype=np.float32)
    masks_u8 = np.ascontiguousarray(np.asarray(masks).reshape(B, S)).view(np.uint8)
    # [B, S, E] -> [B, NT, TT, NE, P] -> [B, NT, NE, P, TT]
    encT = np.ascontiguousarray(
        enc_output.reshape(B, NT, TT, NE, P).transpose(0, 1, 3, 4, 2)
    ).astype(ml_dtypes.bfloat16)
    shared = {
        "w_enc": np.asarray(W_enc, dtype=np.float32),
        "b_enc": np.asarray(b_enc, dtype=np.float32).reshape(A),
        "w_dec": np.asarray(W_dec, dtype=np.float32),
        "b_dec": np.asarray(b_dec, dtype=np.float32).reshape(A),
        "w_att": np.asarray(W_att, dtype=np.float32).reshape(A),
    }
    in_maps = []
    for c in range(NCORES):
        sl = slice(c * BLOC, (c + 1) * BLOC)
        in_maps.append({
            "encT": encT[sl],
            "dec": decoder_hidden[sl],
            "masks": masks_u8[sl],
            **shared,
        })
    return in_maps


def kernel(enc_output, decoder_hidden, masks, W_enc, b_enc, W_dec, b_dec,
           W_att, b_att, **kwargs):
    # b_att shifts every score equally -> cancels in softmax; output does not
    # depend on it, so it is not shipped to the device.
    in_maps = host_prep(enc_output, decoder_hidden, masks, W_enc, b_enc,
                        W_dec, b_dec, W_att, b_att)
    res = run_bass_kernel_spmd(_get_nc(), in_maps, core_ids=list(range(NCORES)))
    return np.concatenate([res.results[c]["out"] for c in range(NCORES)], axis=0)


# revision 26
# speedup vs baseline: 1.2977x; 1.0293x over previous
"""Trainium2 Bass kernel for nn_Attn attention-context module.

Computation (per batch b):
    enc_att = enc @ W_enc + b_enc                      # [S, A]
    dec_att = dec @ W_dec + b_dec                      # [A]
    scores  = tanh(enc_att + dec_att) @ W_att + b_att  # [S]
    w       = softmax(mask(scores))                    # over S
    out     = sum_s w[s] * enc_att[s]                  # [A]

Strategy: data-parallel over batch across 8 NeuronCores (4 batches each),
weights replicated. enc is pre-transposed and cast to bf16 on the host
(layout [b, t, i, p, n] = enc[b, t*TT+n, i*128+p]) so each core streams
contiguous, already-transposed bf16 tiles straight from HBM -- no on-device
transpose pass and half the HBM traffic of fp32. Per core:
  - PE computes enc_attT chunks [A-chunk(128), TT tok] in PSUM (bf16 in,
    fp32 acc)
  - ACT applies tanh (bf16 out) with per-partition bias = dec_att + b_enc,
    and copies raw enc_att to SBUF fp32 for the context accumulation
  - scores via small bf16 PE matmuls with lhsT = W_att chunks; the mask is
    folded in as a -32768*mask K=1 matmul term (exp then underflows to 0)
  - softmax without max-subtraction (|scores| <= ||W_att||_1 ~ 51, exp can't
    overflow fp32; b_att cancels in the softmax so it is dropped)
  - context accumulated per tile with fused DVE multiply+row-sum
    (scalar_tensor_tensor with accum_out) against a broadcast row of softmax
    numerators; normalization and b_enc are applied once per batch
"""

import os
import sys

import numpy as np

for _p in ("/opt/trn_rl_repo", "/root/.axon_site/_ro/trn_rl_repo"):
    if os.path.isdir(_p) and _p not in sys.path:
        sys.path.append(_p)

import concourse.bass as bass
import bass_rust
import concourse.mybir as mybir
from concourse import tile
from concourse.bass_utils import run_bass_kernel_spmd

P = 128
E = 1024          # 2*HIDDEN
A = 512           # ATT
HID = 512
S = 2048
B = 32
NCORES = 8
BLOC = B // NCORES           # 4 batches per core
TT = 512                     # tokens per softmax/context tile
NT = S // TT                 # 4 logical tiles per batch
ST = 1024                    # tokens per main-matmul supertile (bf16 moving
                             # operand max is 128x1024)
NST = S // ST                # 2 supertiles per batch
NE = E // P                  # 8 E-chunks
NA = A // P                  # 4 A-chunks
NK = TT // P                 # 4 token blocks per tile

f32 = mybir.dt.float32
bf16 = mybir.dt.bfloat16
u8 = mybir.dt.uint8

_CACHE = {}


def _split_multiwaits(nc):
    """This toolchain's walrus encodes at most 1 sync-wait per instruction
    (2 for EventSemaphore). Hoist extra waits onto pure-wait EventSemaphore
    instructions inserted immediately before the offender (same engine), which
    preserves semantics exactly."""
    n_split = 0
    uid = 0
    for fn in nc.m.functions:
        for blk in fn.blocks:
            new_insts = []
            for inst in blk.instructions:
                cap = 2 if type(inst).__name__ == "InstEventSemaphore" else 1
                si = inst.sync_info
                waits = list(si.on_wait) if si is not None and si.on_wait else []
                if len(waits) > cap:
                    extra, keep = waits[:-cap], waits[-cap:]
                    for i in range(0, len(extra), 2):
                        uid += 1
                        new_insts.append(bass_rust.InstEventSemaphore(
                            name=f"splitwait_{uid}_{inst.name}",
                            engine=inst.engine,
                            ins=[],
                            outs=[],
                            sync_info=bass_rust.SyncInfo(
                                on_wait=list(extra[i:i + 2]), on_update=[]),
                        ))
                        n_split += 1
                    si.on_wait = keep
                new_insts.append(inst)
            blk.instructions[:] = new_insts
    return n_split


def build(encbufs=3, reps=1):
    nc = bass.Bass("TRN2", debug=False)
    # host-pre-transposed bf16 enc: [b, T, i, p, n] = enc[b, T*ST+n, i*P+p]
    encT = nc.dram_tensor("encT", [BLOC, NST, NE, P, ST], bf16,
                          kind="ExternalInput")
    dec = nc.dram_tensor("dec", [BLOC, HID], f32, kind="ExternalInput")
    masks = nc.dram_tensor("masks", [BLOC, S], u8, kind="ExternalInput")
    w_enc = nc.dram_tensor("w_enc", [E, A], f32, kind="ExternalInput")
    b_enc = nc.dram_tensor("b_enc", [A], f32, kind="ExternalInput")
    w_dec = nc.dram_tensor("w_dec", [HID, A], f32, kind="ExternalInput")
    b_dec = nc.dram_tensor("b_dec", [A], f32, kind="ExternalInput")
    w_att = nc.dram_tensor("w_att", [A], f32, kind="ExternalInput")
    out = nc.dram_tensor("out", [BLOC, A], f32, kind="ExternalOutput")

    Tanh = mybir.ActivationFunctionType.Tanh
    Exp = mybir.ActivationFunctionType.Exp
    Copy = mybir.ActivationFunctionType.Copy
    add = mybir.AluOpType.add
    mult = mybir.AluOpType.mult
    X = mybir.AxisListType.X

    with tile.TileContext(nc) as tc:
        with (
            tc.tile_pool(name="const", bufs=1) as cp,
            tc.tile_pool(name="encT", bufs=encbufs) as encp,
            tc.tile_pool(name="tanh", bufs=3) as tanhp,
            tc.tile_pool(name="ea", bufs=3) as eap,
            tc.tile_pool(name="small", bufs=3) as smp,
            tc.tile_pool(name="attps", bufs=2, space="PSUM") as attp,
            tc.tile_pool(name="scps", bufs=2, space="PSUM") as scp,
            tc.tile_pool(name="pbps", bufs=2, space="PSUM") as pbp,
        ):
            # ---------------- one-time prep ----------------
            # W_enc bf16: [e' part, (i, a)] for e = i*128 + e'
            wsb = cp.tile([P, NE * A], bf16, tag="wsb")
            nc.gpsimd.dma_start(
                wsb[:].rearrange("p (i a) -> p i a", i=NE),
                w_enc.ap().rearrange("(i p) a -> p i a", p=P))
            # W_dec f32: [h' part, (i, a)] for h = i*128 + h'
            # (one-time loads ride the vector/scalar queues so the sync queue
            # is free for the first encT tile)
            wdsb = cp.tile([P, (HID // P) * A], f32, tag="wdsb")
            nc.scalar.dma_start(
                wdsb[:].rearrange("p (i a) -> p i a", i=HID // P),
                w_dec.ap().rearrange("(i p) a -> p i a", p=P))
            # W_att bf16 column chunks [a' part, j]
            wasb = cp.tile([P, NA], bf16, tag="wasb")
            nc.gpsimd.dma_start(wasb[:], w_att.ap().rearrange("(j p) -> p j", p=P))
            # biases as column chunks [a' part, j]
            besb = cp.tile([P, NA], f32, tag="besb")
            nc.scalar.dma_start(besb[:], b_enc.ap().rearrange("(j p) -> p j", p=P))
            bdsb = cp.tile([P, NA], f32, tag="bdsb")
            nc.scalar.dma_start(bdsb[:], b_dec.ap().rearrange("(j p) -> p j", p=P))
            bbsb = cp.tile([P, NA], f32, tag="bbsb")
            nc.vector.tensor_tensor(bbsb[:], besb[:], bdsb[:], op=add)
            # decoder_hidden transposed [h' part, (hc, b)] (tiny strided load)
            dhT = cp.tile([P, (HID // P) * BLOC], f32, tag="dhT")
            with nc.allow_non_contiguous_dma(reason="8KB one-time transposed load"):
                for hc in range(HID // P):
                    nc.scalar.dma_start(
                        dhT[:, hc * BLOC:(hc + 1) * BLOC],
                        dec.ap()[:, hc * P:(hc + 1) * P].rearrange("b p -> p b"))
            # masks, whole core's worth: [1, BLOC*S] u8 -> bf16
            msku = cp.tile([1, BLOC * S], u8, tag="msku")
            nc.scalar.dma_start(msku[:], masks.ap().rearrange("b s -> (b s)")[None, :])
            mskf = cp.tile([1, BLOC * S], bf16, tag="mskf")
            nc.vector.tensor_copy(mskf[:], msku[:])
            # ones row (for partition broadcast via K=1 matmul)
            ones = cp.tile([1, P], bf16, tag="ones")
            nc.vector.memset(ones[:], 1.0)
            onesf = cp.tile([1, P], f32, tag="onesf")
            nc.vector.memset(onesf[:], 1.0)
            # mask weight for folding -32768*mask into the scores matmul
            m30 = cp.tile([1, 1], bf16, tag="m30")
            nc.vector.memset(m30[:], -32768.0)

            # dec_attT + bias columns: bias_sb[a', j*BLOC + b]
            bias_sb = cp.tile([P, NA * BLOC], f32, tag="bias_sb")
            for j in range(NA):
                pd = pbp.tile([P, BLOC], f32, tag="pb")
                for hc in range(HID // P):
                    nc.tensor.matmul(
                        pd[:],
                        lhsT=wdsb[:, hc * A + j * P: hc * A + (j + 1) * P],
                        rhs=dhT[:, hc * BLOC:(hc + 1) * BLOC],
                        start=(hc == 0), stop=(hc == HID // P - 1))
                nc.vector.tensor_scalar(
                    out=bias_sb[:, j * BLOC:(j + 1) * BLOC], in0=pd[:],
                    scalar1=bbsb[:, j:j + 1], scalar2=None, op0=add)

            # persistent accumulators
            ctxp = cp.tile([P, NA * NT], f32, tag="ctxp")       # per (A-chunk, tile) partial
            ctxs = cp.tile([P, NA], f32, tag="ctxs")
            dens = cp.tile([1, BLOC * NT], f32, tag="dens")     # per-tile denominators
            dent = cp.tile([1, BLOC], f32, tag="dent")
            rec = cp.tile([1, BLOC], f32, tag="rec")
            outsb = cp.tile([P, NA * BLOC], f32, tag="outsb")

            # ---------------- main loop ----------------
            # Per-tile epilogues (scores -> softmax -> context) are deferred
            # until after the NEXT tile's main matmuls are emitted, so the PE
            # queue always has dense main-matmul work between epilogue stalls.
            pending = []

            def tile_epilogue(b, t, ea_sb, tanh_sb):
                # t is the logical TT-token tile index; tanh_sb/ea_sb hold a
                # full ST-token supertile with column layout (j, ST)
                bt = b * NT + t
                h = t % (ST // TT)
                # scores [1, TT]; mask folded in as a K=1 matmul term
                sc = scp.tile([1, TT], f32, tag="sc")
                for j in range(NA):
                    nc.tensor.matmul(
                        sc[:], lhsT=wasb[:, j:j + 1],
                        rhs=tanh_sb[:, j * ST + h * TT:j * ST + (h + 1) * TT],
                        start=(j == 0), stop=False)
                nc.tensor.matmul(
                    sc[:], lhsT=m30[:],
                    rhs=mskf[0:1, (b * S + t * TT):(b * S + (t + 1) * TT)],
                    start=False, stop=True)
                p_sb = smp.tile([1, TT], bf16, tag="p_sb")
                nc.scalar.activation(
                    p_sb[:], sc[:], Exp,
                    accum_out=dens[0:1, bt:bt + 1])
                # broadcast p across partitions via K=1 matmul
                pb = pbp.tile([P, TT], f32, tag="pb")
                nc.tensor.matmul(pb[:], lhsT=ones[:], rhs=p_sb[:])
                # fused context accumulation per A-chunk:
                # accum_out = sum_t(pb * ea) per partition
                waste = smp.tile([P, TT], bf16, tag="waste")
                for j in range(NA):
                    nc.vector.scalar_tensor_tensor(
                        out=waste[:],
                        in0=pb[:], scalar=1.0,
                        in1=ea_sb[:, j * ST + h * TT:j * ST + (h + 1) * TT],
                        op0=mult, op1=mult,
                        accum_out=ctxp[:, j * NT + t:j * NT + t + 1])

            def batch_epilogue(b):
                # out[b] = ctx/den + b_enc
                nc.vector.reduce_sum(
                    dent[0:1, b:b + 1], dens[0:1, b * NT:(b + 1) * NT], axis=X)
                nc.vector.reciprocal(rec[0:1, b:b + 1], dent[0:1, b:b + 1])
                rb = pbp.tile([P, TT], f32, tag="pb")
                nc.tensor.matmul(rb[:, 0:1], lhsT=onesf[:], rhs=rec[0:1, b:b + 1])
                nc.vector.reduce_sum(
                    ctxs[:], ctxp[:].rearrange("p (j t) -> p j t", j=NA), axis=X)
                nc.vector.scalar_tensor_tensor(
                    out=outsb[:, b * NA:(b + 1) * NA],
                    in0=ctxs[:], scalar=rb[:, 0:1], in1=besb[:], op0=mult, op1=add)
                nc.gpsimd.dma_start(
                    out.ap()[b].rearrange("(j p) -> p j", p=P),
                    outsb[:, b * NA:(b + 1) * NA])

            for b, T in [(bb_, TT_) for _ in range(reps)
                         for bb_ in range(BLOC) for TT_ in range(NST)]:
                # load pre-transposed supertile: encTs[e', (i, n)]
                encTs = encp.tile([P, NE * ST], bf16, tag="encT")
                nc.sync.dma_start(
                    encTs[:].rearrange("p (i n) -> p i n", i=NE),
                    encT.ap()[b, T].rearrange("i p n -> p i n"))

                tanh_sb = tanhp.tile([P, NA * ST], bf16, tag="tanh")
                ea_sb = eap.tile([P, NA * ST], f32, tag="ea")
                for j in range(NA):
                    att = attp.tile([P, ST], f32, tag="att")
                    for i in range(NE):
                        # two bank-sized matmuls per (i, j); consecutive MMs
                        # share the same stationary operand
                        for h in range(2):
                            nc.tensor.matmul(
                                att[:, h * TT:(h + 1) * TT],
                                lhsT=wsb[:, i * A + j * P: i * A + (j + 1) * P],
                                rhs=encTs[:, i * ST + h * TT:i * ST + (h + 1) * TT],
                                start=(i == 0), stop=(i == NE - 1))
                    nc.scalar.activation(
                        tanh_sb[:, j * ST:(j + 1) * ST], att[:],
                        Tanh, bias=bias_sb[:, j * BLOC + b: j * BLOC + b + 1])
                    # copy enc_att out of PSUM (frees the bank for the next
                    # supertile's matmuls): ACT for the first pair, DVE for
                    # the second, to balance engine load.
                    if j < 2:
                        nc.scalar.activation(
                            ea_sb[:, j * ST:(j + 1) * ST], att[:], Copy)
                    else:
                        nc.vector.tensor_copy(
                            ea_sb[:, j * ST:(j + 1) * ST], att[:])

                while len(pending) >= 4:
                    pending.pop(0)()
                for h in range(ST // TT):
                    pending.append(
                        lambda b=b, t=T * (ST // TT) + h, a=ea_sb, ts=tanh_sb:
                        tile_epilogue(b, t, a, ts))
                if T == NST - 1:
                    # batch epilogue rides the deferred queue too, so the
                    # next batch's main matmuls keep the PE fed while this
                    # batch's softmax/context tail drains
                    pending.append(lambda b=b: batch_epilogue(b))
            for fn_ in pending:
                fn_()

    n = _split_multiwaits(nc)
    if os.environ.get("KERNEL_DEBUG"):
        print(f"[kernel] split {n} extra waits", file=sys.stderr)
    return nc


def _get_nc():
    if "nc" not in _CACHE:
        _CACHE["nc"] = build()
    return _CACHE["nc"]


def host_prep(enc_output, decoder_hidden, masks, W_enc, b_enc, W_dec, b_dec,
              W_att, b_att=None, **kwargs):
    """Shard + lay out inputs for the 8 cores. enc is cast to bf16 and
    pre-transposed to [b, t, i, p, n] = enc[b, t*TT+n, i*P+p] so the device
    streams contiguous already-transposed tiles."""
    import ml_dtypes

    enc_output = np.asarray(enc_output, dtype=np.float32)
    decoder_hidden = np.asarray(decoder_hidden, dtype=np.float32)
    masks_u8 = np.ascontiguousarray(np.asarray(masks).reshape(B, S)).view(np.uint8)
    # [B, S, E] -> [B, NST, ST, NE, P] -> [B, NST, NE, P, ST]
    encT = np.ascontiguousarray(
        enc_output.reshape(B, NST, ST, NE, P).transpose(0, 1, 3, 4, 2)
    ).astype(ml_dtypes.bfloat16)
    shared = {
        "w_enc": np.asarray(W_enc, dtype=np.float32),
        "b_enc": np.asarray(b_enc, dtype=np.float32).reshape(A),
        "w_dec": np.asarray(W_dec, dtype=np.float32),
        "b_dec": np.asarray(b_dec, dtype=np.float32).reshape(A),
        "w_att": np.asarray(W_att, dtype=np.float32).reshape(A),
    }
    in_maps = []
    for c in range(NCORES):
        sl = slice(c * BLOC, (c + 1) * BLOC)
        in_maps.append({
            "encT": encT[sl],
            "dec": decoder_hidden[sl],
            "masks": masks_u8[sl],
            **shared,
        })
    return in_maps


def kernel(enc_output, decoder_hidden, masks, W_enc, b_enc, W_dec, b_dec,
           W_att, b_att, **kwargs):
    # b_att shifts every score equally -> cancels in softmax; output does not
    # depend on it, so it is not shipped to the device.
    in_maps = host_prep(enc_output, decoder_hidden, masks, W_enc, b_enc,
                        W_dec, b_dec, W_att, b_att)
    res = run_bass_kernel_spmd(_get_nc(), in_maps, core_ids=list(range(NCORES)))
    return np.concatenate([res.results[c]["out"] for c in range(NCORES)], axis=0)


# revision 27
# speedup vs baseline: 1.5692x; 1.2092x over previous
"""Trainium2 Bass kernel for nn_Attn attention-context module.

Computation (per batch b):
    enc_att = enc @ W_enc + b_enc                      # [S, A]
    dec_att = dec @ W_dec + b_dec                      # [A]
    scores  = tanh(enc_att + dec_att) @ W_att + b_att  # [S]
    w       = softmax(mask(scores))                    # over S
    out     = sum_s w[s] * enc_att[s]                  # [A]

Strategy: data-parallel over batch across 8 NeuronCores (4 batches each),
weights replicated.

Masked tokens contribute exactly zero to the softmax numerator, denominator
and context (their score gets -32768 folded in, and exp underflows to +0), so
the host compacts each batch to its unmasked tokens, padded with zeroed,
masked-out tokens up to a global per-batch token count Kp (multiple of 128,
shared by all batches so the 8 cores run one instruction stream). With the
reference's ~50% mask density this roughly halves all device work. The
compacted enc is cast to bf16 and pre-transposed on the host (layout
[b, slot, i, p, n] = enc_c[b, slot*TT+n, i*128+p]) so each core streams
contiguous, already-transposed bf16 tiles straight from HBM.

Per core:
  - PE computes enc_attT chunks [A-chunk(128), W tok] in PSUM (bf16 in,
    fp32 acc)
  - ACT applies tanh (bf16 out) with per-partition bias = dec_att + b_enc,
    and copies raw enc_att to SBUF fp32 for the context accumulation
  - scores via small bf16 PE matmuls with lhsT = W_att chunks; the mask is
    folded in as a -32768*mask K=1 matmul term (exp then underflows to 0,
    killing the padding tokens)
  - softmax without max-subtraction (|scores| <= ||W_att||_1 ~ 51, exp can't
    overflow fp32; b_att cancels in the softmax so it is dropped)
  - context accumulated per tile with fused DVE multiply+row-sum
    (scalar_tensor_tensor with accum_out) against a broadcast row of softmax
    numerators; normalization and b_enc are applied once per batch
"""

import os
import sys

import numpy as np

for _p in ("/opt/trn_rl_repo", "/root/.axon_site/_ro/trn_rl_repo"):
    if os.path.isdir(_p) and _p not in sys.path:
        sys.path.append(_p)

import concourse.bass as bass
import bass_rust
import concourse.mybir as mybir
from concourse import tile
from concourse.bass_utils import run_bass_kernel_spmd

P = 128
E = 1024          # 2*HIDDEN
A = 512           # ATT
HID = 512
S = 2048
B = 32
NCORES = 8
BLOC = B // NCORES           # 4 batches per core
TT = 512                     # max tokens per tile
NE = E // P                  # 8 E-chunks
NA = A // P                  # 4 A-chunks

f32 = mybir.dt.float32
bf16 = mybir.dt.bfloat16
u8 = mybir.dt.uint8

_CACHE = {}


def tile_widths(kp):
    """Split the padded per-batch token count into PE-tile widths."""
    assert kp % P == 0 and kp >= P
    w = [TT] * (kp // TT)
    if kp % TT:
        w.append(kp % TT)
    return w


def _split_multiwaits(nc):
    """This toolchain's walrus encodes at most 1 sync-wait per instruction
    (2 for EventSemaphore). Hoist extra waits onto pure-wait EventSemaphore
    instructions inserted immediately before the offender (same engine), which
    preserves semantics exactly."""
    n_split = 0
    uid = 0
    for fn in nc.m.functions:
        for blk in fn.blocks:
            new_insts = []
            for inst in blk.instructions:
                cap = 2 if type(inst).__name__ == "InstEventSemaphore" else 1
                si = inst.sync_info
                waits = list(si.on_wait) if si is not None and si.on_wait else []
                if len(waits) > cap:
                    extra, keep = waits[:-cap], waits[-cap:]
                    for i in range(0, len(extra), 2):
                        uid += 1
                        new_insts.append(bass_rust.InstEventSemaphore(
                            name=f"splitwait_{uid}_{inst.name}",
                            engine=inst.engine,
                            ins=[],
                            outs=[],
                            sync_info=bass_rust.SyncInfo(
                                on_wait=list(extra[i:i + 2]), on_update=[]),
                        ))
                        n_split += 1
                    si.on_wait = keep
                new_insts.append(inst)
            blk.instructions[:] = new_insts
    return n_split


def build(kp=S, encbufs=3, reps=1):
    widths = tile_widths(kp)
    NSLOT = len(widths)

    nc = bass.Bass("TRN2", debug=False)
    # host-compacted, pre-transposed bf16 enc:
    # [b, slot, i, p, n] = enc_compact[b, slot*TT+n, i*P+p]
    encT = nc.dram_tensor("encT", [BLOC, NSLOT, NE, P, TT], bf16,
                          kind="ExternalInput")
    dec = nc.dram_tensor("dec", [BLOC, HID], f32, kind="ExternalInput")
    # compacted masks, padded region = 1: [b, slot*TT + n]
    masks = nc.dram_tensor("masks", [BLOC, NSLOT * TT], u8,
                           kind="ExternalInput")
    w_enc = nc.dram_tensor("w_enc", [E, A], f32, kind="ExternalInput")
    b_enc = nc.dram_tensor("b_enc", [A], f32, kind="ExternalInput")
    w_dec = nc.dram_tensor("w_dec", [HID, A], f32, kind="ExternalInput")
    b_dec = nc.dram_tensor("b_dec", [A], f32, kind="ExternalInput")
    w_att = nc.dram_tensor("w_att", [A], f32, kind="ExternalInput")
    out = nc.dram_tensor("out", [BLOC, A], f32, kind="ExternalOutput")

    Tanh = mybir.ActivationFunctionType.Tanh
    Exp = mybir.ActivationFunctionType.Exp
    Copy = mybir.ActivationFunctionType.Copy
    add = mybir.AluOpType.add
    mult = mybir.AluOpType.mult
    X = mybir.AxisListType.X

    with tile.TileContext(nc) as tc:
        with (
            tc.tile_pool(name="const", bufs=1) as cp,
            tc.tile_pool(name="encT", bufs=encbufs) as encp,
            tc.tile_pool(name="tanh", bufs=4) as tanhp,
            tc.tile_pool(name="ea", bufs=4) as eap,
            tc.tile_pool(name="small", bufs=3) as smp,
            tc.tile_pool(name="attps", bufs=2, space="PSUM") as attp,
            tc.tile_pool(name="scps", bufs=2, space="PSUM") as scp,
            tc.tile_pool(name="pbps", bufs=2, space="PSUM") as pbp,
        ):
            # ---------------- one-time prep ----------------
            # W_enc bf16: [e' part, (i, a)] for e = i*128 + e'
            wsb = cp.tile([P, NE * A], bf16, tag="wsb")
            nc.gpsimd.dma_start(
                wsb[:].rearrange("p (i a) -> p i a", i=NE),
                w_enc.ap().rearrange("(i p) a -> p i a", p=P))
            # W_dec f32: [h' part, (i, a)] for h = i*128 + h'
            # (one-time loads ride the scalar/gpsimd queues so the sync queue
            # is free for the first encT tile)
            wdsb = cp.tile([P, (HID // P) * A], f32, tag="wdsb")
            nc.scalar.dma_start(
                wdsb[:].rearrange("p (i a) -> p i a", i=HID // P),
                w_dec.ap().rearrange("(i p) a -> p i a", p=P))
            # W_att bf16 column chunks [a' part, j]
            wasb = cp.tile([P, NA], bf16, tag="wasb")
            nc.gpsimd.dma_start(wasb[:], w_att.ap().rearrange("(j p) -> p j", p=P))
            # biases as column chunks [a' part, j]
            besb = cp.tile([P, NA], f32, tag="besb")
            nc.scalar.dma_start(besb[:], b_enc.ap().rearrange("(j p) -> p j", p=P))
            bdsb = cp.tile([P, NA], f32, tag="bdsb")
            nc.scalar.dma_start(bdsb[:], b_dec.ap().rearrange("(j p) -> p j", p=P))
            bbsb = cp.tile([P, NA], f32, tag="bbsb")
            nc.vector.tensor_tensor(bbsb[:], besb[:], bdsb[:], op=add)
            # decoder_hidden transposed [h' part, (hc, b)] (tiny strided load)
            dhT = cp.tile([P, (HID // P) * BLOC], f32, tag="dhT")
            with nc.allow_non_contiguous_dma(reason="8KB one-time transposed load"):
                for hc in range(HID // P):
                    nc.scalar.dma_start(
                        dhT[:, hc * BLOC:(hc + 1) * BLOC],
                        dec.ap()[:, hc * P:(hc + 1) * P].rearrange("b p -> p b"))
            # masks, whole core's worth: [1, BLOC*NSLOT*TT] u8 -> bf16
            msku = cp.tile([1, BLOC * NSLOT * TT], u8, tag="msku")
            nc.scalar.dma_start(msku[:], masks.ap().rearrange("b s -> (b s)")[None, :])
            mskf = cp.tile([1, BLOC * NSLOT * TT], bf16, tag="mskf")
            nc.vector.tensor_copy(mskf[:], msku[:])
            # ones row (for partition broadcast via K=1 matmul)
            ones = cp.tile([1, P], bf16, tag="ones")
            nc.vector.memset(ones[:], 1.0)
            onesf = cp.tile([1, P], f32, tag="onesf")
            nc.vector.memset(onesf[:], 1.0)
            # mask weight for folding -32768*mask into the scores matmul
            m30 = cp.tile([1, 1], bf16, tag="m30")
            nc.vector.memset(m30[:], -32768.0)

            # dec_attT + bias columns: bias_sb[a', j*BLOC + b]
            bias_sb = cp.tile([P, NA * BLOC], f32, tag="bias_sb")
            for j in range(NA):
                pd = pbp.tile([P, BLOC], f32, tag="pb")
                for hc in range(HID // P):
                    nc.tensor.matmul(
                        pd[:],
                        lhsT=wdsb[:, hc * A + j * P: hc * A + (j + 1) * P],
                        rhs=dhT[:, hc * BLOC:(hc + 1) * BLOC],
                        start=(hc == 0), stop=(hc == HID // P - 1))
                nc.vector.tensor_scalar(
                    out=bias_sb[:, j * BLOC:(j + 1) * BLOC], in0=pd[:],
                    scalar1=bbsb[:, j:j + 1], scalar2=None, op0=add)

            # persistent accumulators
            ctxp = cp.tile([P, NA * NSLOT], f32, tag="ctxp")    # per (A-chunk, slot)
            ctxs = cp.tile([P, NA], f32, tag="ctxs")
            dens = cp.tile([1, BLOC * NSLOT], f32, tag="dens")  # per-slot denominators
            dent = cp.tile([1, BLOC], f32, tag="dent")
            rec = cp.tile([1, BLOC], f32, tag="rec")
            outsb = cp.tile([P, NA * BLOC], f32, tag="outsb")

            # ---------------- main loop ----------------
            # Per-tile epilogues (scores -> softmax -> context) are deferred
            # a few tiles so the PE queue always has dense main-matmul work
            # between epilogue stalls; the batch epilogue rides the same queue.
            pending = []

            def tile_epilogue(b, t, w, ea_sb, tanh_sb):
                bt = b * NSLOT + t
                # scores [1, w]; mask folded in as a K=1 matmul term
                sc = scp.tile([1, TT], f32, tag="sc")
                for j in range(NA):
                    nc.tensor.matmul(
                        sc[:, :w], lhsT=wasb[:, j:j + 1],
                        rhs=tanh_sb[:, j * w:(j + 1) * w],
                        start=(j == 0), stop=False)
                nc.tensor.matmul(
                    sc[:, :w], lhsT=m30[:],
                    rhs=mskf[0:1, bt * TT: bt * TT + w],
                    start=False, stop=True)
                p_sb = smp.tile([1, TT], bf16, tag="p_sb")
                nc.scalar.activation(
                    p_sb[:, :w], sc[:, :w], Exp,
                    accum_out=dens[0:1, bt:bt + 1])
                # broadcast p across partitions via K=1 matmul
                pb = pbp.tile([P, TT], f32, tag="pb")
                nc.tensor.matmul(pb[:, :w], lhsT=ones[:], rhs=p_sb[:, :w])
                # fused context accumulation per A-chunk:
                # accum_out = sum_tok(pb * ea) per partition
                waste = smp.tile([P, TT], bf16, tag="waste")
                for j in range(NA):
                    nc.vector.scalar_tensor_tensor(
                        out=waste[:, :w],
                        in0=pb[:, :w], scalar=1.0, in1=ea_sb[:, j * w:(j + 1) * w],
                        op0=mult, op1=mult,
                        accum_out=ctxp[:, j * NSLOT + t:j * NSLOT + t + 1])

            def batch_epilogue(b):
                # out[b] = ctx/den + b_enc
                nc.vector.reduce_sum(
                    dent[0:1, b:b + 1], dens[0:1, b * NSLOT:(b + 1) * NSLOT],
                    axis=X)
                nc.vector.reciprocal(rec[0:1, b:b + 1], dent[0:1, b:b + 1])
                rb = pbp.tile([P, TT], f32, tag="pb")
                nc.tensor.matmul(rb[:, 0:1], lhsT=onesf[:], rhs=rec[0:1, b:b + 1])
                nc.vector.reduce_sum(
                    ctxs[:], ctxp[:].rearrange("p (j t) -> p j t", j=NA), axis=X)
                nc.vector.scalar_tensor_tensor(
                    out=outsb[:, b * NA:(b + 1) * NA],
                    in0=ctxs[:], scalar=rb[:, 0:1], in1=besb[:], op0=mult, op1=add)
                nc.gpsimd.dma_start(
                    out.ap()[b].rearrange("(j p) -> p j", p=P),
                    outsb[:, b * NA:(b + 1) * NA])

            for b, t in [(bb_, tt_) for _ in range(reps)
                         for bb_ in range(BLOC) for tt_ in range(NSLOT)]:
                w = widths[t]
                # load pre-transposed tile: encTt[e', (i, n)], n < w
                encTt = encp.tile([P, NE * TT], bf16, tag="encT")
                nc.sync.dma_start(
                    encTt[:].rearrange("p (i n) -> p i n", i=NE)[:, :, :w],
                    encT.ap()[b, t].rearrange("i p n -> p i n")[:, :, :w])

                tanh_sb = tanhp.tile([P, NA * TT], bf16, tag="tanh")
                ea_sb = eap.tile([P, NA * TT], f32, tag="ea")
                for j in range(NA):
                    att = attp.tile([P, TT], f32, tag="att")
                    for i in range(NE):
                        nc.tensor.matmul(
                            att[:, :w],
                            lhsT=wsb[:, i * A + j * P: i * A + (j + 1) * P],
                            rhs=encTt[:, i * TT:i * TT + w],
                            start=(i == 0), stop=(i == NE - 1))
                    nc.scalar.activation(
                        tanh_sb[:, j * w:(j + 1) * w], att[:, :w],
                        Tanh, bias=bias_sb[:, j * BLOC + b: j * BLOC + b + 1])
                    # copy enc_att out of PSUM (frees the bank for the next
                    # tile's matmuls): ACT for the first pair, DVE for the
                    # second, to balance engine load.
                    if j < 2:
                        nc.scalar.activation(
                            ea_sb[:, j * w:(j + 1) * w], att[:, :w], Copy)
                    else:
                        nc.vector.tensor_copy(
                            ea_sb[:, j * w:(j + 1) * w], att[:, :w])

                while len(pending) >= 3:
                    pending.pop(0)()
                pending.append(
                    lambda b=b, t=t, w=w, a=ea_sb, ts=tanh_sb:
                    tile_epilogue(b, t, w, a, ts))
                if t == NSLOT - 1:
                    pending.append(lambda b=b: batch_epilogue(b))
            for fn_ in pending:
                fn_()

    n = _split_multiwaits(nc)
    if os.environ.get("KERNEL_DEBUG"):
        print(f"[kernel] split {n} extra waits", file=sys.stderr)
    return nc


def _get_nc(kp):
    if kp not in _CACHE:
        _CACHE[kp] = build(kp=kp)
    return _CACHE[kp]


def kp_from_masks(masks):
    """Global padded per-batch token count: max unmasked count over all
    batches, rounded up to a multiple of 128."""
    m = np.asarray(masks).reshape(B, S)
    kmax = int((~m.astype(bool)).sum(axis=1).max())
    kmax = max(kmax, P)
    return -(-kmax // P) * P


def host_prep(enc_output, decoder_hidden, masks, W_enc, b_enc, W_dec, b_dec,
              W_att, b_att=None, kp=None, **kwargs):
    """Shard + lay out inputs for the 8 cores: compact each batch to its
    unmasked tokens (padded to kp with zeroed masked tokens), cast enc to
    bf16 and pre-transpose to [b, slot, i, p, n] = enc_c[b, slot*TT+n, i*P+p]
    so the device streams contiguous already-transposed tiles."""
    import ml_dtypes

    enc_output = np.asarray(enc_output, dtype=np.float32)
    decoder_hidden = np.asarray(decoder_hidden, dtype=np.float32)
    masks_b = np.asarray(masks).reshape(B, S).astype(bool)
    if kp is None:
        kp = kp_from_masks(masks_b)
    nslot = len(tile_widths(kp))
    kpad = nslot * TT

    enc_c = np.zeros((B, kpad, E), dtype=np.float32)
    mask_c = np.ones((B, kpad), dtype=np.uint8)
    for b in range(B):
        idx = np.flatnonzero(~masks_b[b])[:kp]
        enc_c[b, :len(idx)] = enc_output[b, idx]
        mask_c[b, :len(idx)] = 0
    # [B, kpad, E] -> [B, NSLOT, TT, NE, P] -> [B, NSLOT, NE, P, TT]
    encT = np.ascontiguousarray(
        enc_c.reshape(B, nslot, TT, NE, P).transpose(0, 1, 3, 4, 2)
    ).astype(ml_dtypes.bfloat16)

    shared = {
        "w_enc": np.asarray(W_enc, dtype=np.float32),
        "b_enc": np.asarray(b_enc, dtype=np.float32).reshape(A),
        "w_dec": np.asarray(W_dec, dtype=np.float32),
        "b_dec": np.asarray(b_dec, dtype=np.float32).reshape(A),
        "w_att": np.asarray(W_att, dtype=np.float32).reshape(A),
    }
    in_maps = []
    for c in range(NCORES):
        sl = slice(c * BLOC, (c + 1) * BLOC)
        in_maps.append({
            "encT": encT[sl],
            "dec": decoder_hidden[sl],
            "masks": mask_c[sl],
            **shared,
        })
    return in_maps, kp


def kernel(enc_output, decoder_hidden, masks, W_enc, b_enc, W_dec, b_dec,
           W_att, b_att, **kwargs):
    # b_att shifts every score equally -> cancels in softmax; output does not
    # depend on it, so it is not shipped to the device.
    in_maps, kp = host_prep(enc_output, decoder_hidden, masks, W_enc, b_enc,
                            W_dec, b_dec, W_att, b_att)
    res = run_bass_kernel_spmd(_get_nc(kp), in_maps, core_ids=list(range(NCORES)))
    return np.concatenate([res.results[c]["out"] for c in range(NCORES)], axis=0)
